# revision 18
# baseline (speedup 1.0000x reference)
"""Head-sharded causal self-attention (QK-RMSNorm + RoPE + value-residual mix)
for 8 Trainium2 NeuronCores.

Sharding: 16 heads -> 2 heads per core (tensor parallel). Each core computes
its heads' QKV projections, attention, and a partial c_proj output
[D, T] (transposed, bf16); the host sums the 8 partials (the c_proj
all-reduce).

Layout strategy (per core):
 - QKV matmul produces q,k,v in natural [T, hd] tiles (lhsT = x^T tiles,
   moving = W_all^T), so RMS-norm + RoPE run with T on partitions.
 - q,k are PE-transposed to [hd, T] for the score matmuls.
 - Scores are computed transposed: S^T[T_k, T_q] = k^T_tile.T @ q^T. The
   exp(S^T) tiles (bf16) then directly serve as the moving operand of the
   o^T = v.T @ expS accumulation, so no attention-weight transpose is
   ever needed.
 - Softmax denominator: exp tiles are accumulated per (head, q-block) with
   a chain of bf16 adds (DVE, with every 4th on GpSimd), then ONE
   [1 x QB] ones-matmul per (head, q-block) partition-sums the
   accumulator.  This removes the per-k-tile [128,1,512] PE matmuls the
   previous version spent ~70us on.
 - Causal mask: diagonal-band k-tiles get their score/exp/AV work NARROWED
   to the valid column range; only the [128,128] triangle block needs a
   mask multiply (GpSimd, 0/1 bf16 triangle).
 - o^T[hd, T_q] (normalized, bf16) feeds c_proj: partial^T = Wp^T.T @ o^T.
Matmuls run in float32r for x/w/q/k (accuracy) and bf16 for exp/v/proj
moving operands (1 cycle/row either way; bf16 halves SBUF/DMA traffic).
"""

import numpy as np

import concourse.bacc as bacc
import concourse.mybir as mybir
import concourse.tile as tile
from concourse.bass_utils import run_bass_kernel_spmd

P = 128
T = 4096
D = 2048
HD = 128
NH = 16
HPC = 2            # heads per core
NCORES = 8
NT = T // P        # 32 t-tiles
KT = D // P        # 16 contraction tiles for the projections
NJ = 8             # q-blocks
QB = 512           # q-block width
EPS = 1.1920929e-07

F32 = mybir.dt.float32
F32R = mybir.dt.float32r
BF16 = mybir.dt.bfloat16
MULT = mybir.AluOpType.mult
EXP = mybir.ActivationFunctionType.Exp


def _build():
    nc = bacc.Bacc("TRN2", target_bir_lowering=False, debug=False,
                   enable_asserts=False, num_devices=NCORES)

    # ---- DRAM parameters (host pre-tiled layouts) ----
    xt = nc.dram_tensor("xt", [NT, P, KT, P], BF16, kind="ExternalInput").ap()
    wall = nc.dram_tensor("wall", [P, KT, 6 * HD], BF16, kind="ExternalInput").ap()
    wproj = nc.dram_tensor("wproj", [P, HPC, D], BF16, kind="ExternalInput").ap()
    vilam = nc.dram_tensor("vilam", [NT, P, HPC * HD], F32, kind="ExternalInput").ap()
    cs = nc.dram_tensor("cs", [P, NT, HD], F32, kind="ExternalInput").ap()
    tri = nc.dram_tensor("tri", [P, P], BF16, kind="ExternalInput").ap()
    identr = nc.dram_tensor("identr", [P, P], BF16, kind="ExternalInput").ap()
    out = nc.dram_tensor("out", [D, T], BF16, kind="ExternalOutput").ap()

    with tile.TileContext(nc) as tc:
        with tc.tile_pool(name="persist", bufs=1) as persist:
            # tensors that live for the whole kernel
            qT = persist.tile([P, HPC, T], BF16)        # q^T per head
            kT_ = persist.tile([P, HPC, T], BF16)       # k^T per head
            v_sb = persist.tile([P, HPC, NT, HD], BF16)  # v natural per head
            cs_sbc = [persist.tile([P, 8, HD], F32, name=f"cs_sb{c}")
                      for c in range(4)]
            ident = persist.tile([P, P], BF16)
            tri_bf = persist.tile([P, P], BF16)
            wproj_sb = persist.tile([P, HPC, D], BF16)
            ones_bf = persist.tile([P, 1], BF16)
            eps_q = persist.tile([P, 1], F32)
            eps_k = persist.tile([P, 1], F32)
            nc.gpsimd.memset(eps_q[:], float(P) * EPS)
            nc.gpsimd.memset(eps_k[:], EPS)
            nc.gpsimd.memset(ones_bf[:], 1.0)
            # warm up the gpsimd partition_broadcast ucode path early (its
            # first invocation pays a ~6-7us IRAM load); broadcast is the
            # ONLY op run on GpSimd -- anything else thrashes the Q7 ucode
            # library at ~6us per switch
            warm = persist.tile([P, 8], F32)
            nc.gpsimd.partition_broadcast(warm[:], eps_q[0:1, 0:1]
                                          .broadcast_to([1, 8]))
            # rope outputs of the last two t-tiles persist: their transposes
            # are deferred into phase-2's start so the end of phase 1 does
            # not expose the full ACT->DVE chain latency on the PE queue
            rp_late = [persist.tile([P, 4, P], BF16, name=f"rp_late{i}")
                       for i in range(2)]

            # ---------------- Phase 1: QKV + norm + rope + transposes ----
            with tc.tile_pool(name="p1w", bufs=1) as p1w, \
                 tc.tile_pool(name="p1sb", bufs=4) as p1sb, \
                 tc.tile_pool(name="p1sc", bufs=3) as p1sc, \
                 tc.tile_pool(name="p1ps", bufs=5, space="PSUM") as p1ps, \
                 tc.tile_pool(name="p1tp", bufs=3, space="PSUM") as p1tp:
                # per-(kt,half) weight tiles: fine-grained deps let the first
                # matmuls start as soon as their own chunk has landed
                wall_sb = [[p1w.tile([P, 384], BF16, name=f"wall_sb{kt}_{hf}")
                            for hf in range(2)] for kt in range(KT)]
                # the very first deps: wall lo[0] + first x chunk, issued
                # before anything else so the first matmul starts early
                nc.sync.dma_start(out=wall_sb[0][0][:], in_=wall[:, 0, 0:384])
                x_t0c = [p1w.tile([P, 4, P], BF16, name=f"x_t0c{c}")
                         for c in range(4)]
                nc.sync.dma_start(out=x_t0c[0][:], in_=xt[0, :, 0:4, :])
                nc.sync.dma_start(out=wall_sb[1][0][:], in_=wall[:, 1, 0:384])
                nc.sync.dma_start(out=x_t0c[1][:], in_=xt[0, :, 4:8, :])
                nc.sync.dma_start(out=wall_sb[2][0][:], in_=wall[:, 2, 0:384])
                nc.sync.dma_start(out=x_t0c[2][:], in_=xt[0, :, 8:12, :])
                nc.sync.dma_start(out=wall_sb[3][0][:], in_=wall[:, 3, 0:384])
                nc.sync.dma_start(out=x_t0c[3][:], in_=xt[0, :, 12:16, :])
                nc.sync.dma_start(out=ident[:], in_=identr[:])
                for kt in range(4, KT):
                    nc.sync.dma_start(out=wall_sb[kt][0][:],
                                      in_=wall[:, kt, 0:384])
                vi0 = p1sb.tile([P, HPC * HD], F32, tag="vi", name="vi0")
                nc.sync.dma_start(out=vi0[:], in_=vilam[0])
                for kt in range(KT):
                    nc.sync.dma_start(out=wall_sb[kt][1][:],
                                      in_=wall[:, kt, 384:768])
                x1 = p1sb.tile([P, KT, P], BF16, tag="x", bufs=3, name="x1")
                nc.sync.dma_start(out=x1[:], in_=xt[1])
                nc.sync.dma_start(out=cs_sbc[0][:], in_=cs[:, 0:8, :])

                for tt in range(NT):
                    if 1 <= tt <= 3:
                        nc.sync.dma_start(out=cs_sbc[tt][:],
                                          in_=cs[:, 8 * tt:8 * (tt + 1), :])
                    if tt == 2:
                        nc.sync.dma_start(out=tri_bf[:], in_=tri[:])
                        nc.sync.dma_start(out=wproj_sb[:], in_=wproj[:])
                    if tt == 0:
                        def xop(kt):
                            return x_t0c[kt // 4][:, kt % 4, :]
                    elif tt == 1:
                        def xop(kt, x_t=x1):
                            return x_t[:, kt, :]
                    else:
                        x_t = p1sb.tile([P, KT, P], BF16, tag="x", bufs=3)
                        nc.sync.dma_start(out=x_t[:], in_=xt[tt])

                        def xop(kt, x_t=x_t):
                            return x_t[:, kt, :]
                    if tt == 0:
                        vi_t = vi0
                    else:
                        vi_t = p1sb.tile([P, HPC * HD], F32, tag="vi")
                        nc.sync.dma_start(out=vi_t[:], in_=vilam[tt])
                    halves = []
                    for half in range(2):
                        ps = p1ps.tile([P, 384], F32, tag="qkvps")
                        for kt in range(KT):
                            nc.tensor.matmul(
                                ps[:],
                                xop(kt),
                                wall_sb[kt][half][:],
                                start=(kt == 0),
                                stop=(kt == KT - 1),
                            )
                        halves.append(ps)

                    # --- evict q,k into natural tile + per-row sum-squares
                    qk_nat = p1sb.tile([P, 4, P], F32, tag="qknat")
                    ssq = p1sc.tile([P, 4], F32, tag="ssq")
                    sqs = p1sb.tile([P, P], F32, tag="sqscratch")
                    nc.scalar.copy(qk_nat[:, 0:3, :], halves[0][:, 0:384])
                    nc.scalar.copy(qk_nat[:, 3, :], halves[1][:, 0:P])
                    # squares read the SBUF copy, not PSUM: the qkv psum
                    # then frees right after the copies + v-adds
                    for i in range(4):          # q0 q1 k0 k1
                        nc.scalar.activation(
                            sqs[:], qk_nat[:, i, :],
                            mybir.ActivationFunctionType.Square,
                            accum_out=ssq[:, i:i + 1])
                    # --- v: psum + lam*vi -> bf16 natural tile
                    for h in range(HPC):
                        nc.vector.tensor_add(
                            v_sb[:, h, tt, :],
                            halves[1][:, P + h * P:P + (h + 1) * P],
                            vi_t[:, h * P:(h + 1) * P])

                    # --- rms scales: q -> 1/sqrt(ssq+128eps) (incl 1/sqrt(hd));
                    #     k rows also need a sqrt(128) factor
                    sca = p1sc.tile([P, 4], F32, tag="sca")
                    rsc = p1sc.tile([P, 4], F32, tag="rsc")
                    nc.scalar.activation(sca[:], ssq[:],
                                         mybir.ActivationFunctionType.Sqrt,
                                         bias=eps_q[:], scale=1.0)
                    nc.vector.reciprocal(rsc[:], sca[:])

                    # --- scale q and k by their rms scales
                    for i in range(4):
                        nc.vector.tensor_scalar(
                            out=qk_nat[:, i, :], in0=qk_nat[:, i, :],
                            scalar1=rsc[:, i:i + 1],
                            scalar2=(1.0 if i < 2 else float(np.sqrt(P))),
                            op0=MULT, op1=MULT)

                    # --- rope on all 4 tensors at once (f32r out: final
                    # values ahead of the f32r transpose + score matmuls)
                    if tt >= NT - 2:
                        rp = rp_late[tt - (NT - 2)]
                    else:
                        rp = p1sb.tile([P, 4, P], BF16, tag="rope")
                    tmp = p1sb.tile([P, 4, 64], F32, tag="ropetmp")
                    x1_ = qk_nat[:, :, 0:64]
                    x2 = qk_nat[:, :, 64:128]
                    cst = cs_sbc[tt // 8]
                    cb = cst[:, tt % 8, None, 0:64].broadcast_to([P, 4, 64])
                    sb = cst[:, tt % 8, None, 64:128].broadcast_to([P, 4, 64])
                    nc.vector.tensor_mul(rp[:, :, 0:64], x1_, cb)
                    nc.vector.tensor_mul(tmp[:], x2, sb)
                    nc.vector.tensor_add(rp[:, :, 0:64], rp[:, :, 0:64], tmp[:])
                    nc.vector.tensor_mul(rp[:, :, 64:128], x2, cb)
                    nc.vector.tensor_mul(tmp[:], x1_, sb)
                    nc.vector.tensor_sub(rp[:, :, 64:128], rp[:, :, 64:128], tmp[:])

                    # --- transpose q,k tiles to [hd, T] layout
                    # (paired per tensor: one psum tile + one ACT copy);
                    # the last two tiles are deferred into phase 2
                    if tt < NT - 2:
                        for g, dst in ((0, qT), (1, kT_)):
                            tp = p1tp.tile([P, 2, P], BF16, tag="tp", bufs=3)
                            for i in range(2):
                                nc.tensor.transpose(tp[:, i, :],
                                                    rp[:, 2 * g + i, :],
                                                    ident[:])
                            nc.scalar.copy(dst[:, :, tt * P:(tt + 1) * P],
                                           tp[:])

            # ---------------- Phase 2+3: attention + c_proj ----
            with tc.tile_pool(name="p2exp", bufs=8) as p2exp, \
                 tc.tile_pool(name="p2sb", bufs=4) as p2sb, \
                 tc.tile_pool(name="p2sc", bufs=4) as p2sc, \
                 tc.tile_pool(name="big", bufs=3, space="PSUM") as big, \
                 tc.tile_pool(name="ops", bufs=2, space="PSUM") as ops_:

                def emit_norm(j, o_raw, acc):
                    """softmax-normalize j's raw o into o_sb. o_ps was
                    already released by the o_raw cast, so this chain is
                    latency-tolerant."""
                    o_sb = p2sb.tile([P, HPC, QB], BF16, tag="osb", bufs=3,
                                     name=f"osb_{j}")
                    for h in range(HPC):
                        den = big.tile([1, QB], F32, tag="big",
                                       name=f"den_{j}_{h}")
                        nc.tensor.matmul(den[:], ones_bf[:], acc[:, h, :],
                                         start=True, stop=True)
                        rden = p2sc.tile([1, QB], F32, tag="rden")
                        nc.vector.reciprocal_approx_fast(rden[:], den[:])
                        bc = p2sb.tile([P, QB], F32, tag="bc")
                        nc.gpsimd.partition_broadcast(bc[:], rden[:])
                        nc.vector.tensor_mul(o_sb[:, h, :], o_raw[:, h, :],
                                             bc[:])
                    return o_sb

                def emit_proj_pair(j, o_sb, dtp):
                    """c_proj for two adjacent 128-row output blocks."""
                    pp = big.tile([P, 2, QB], F32, tag="big",
                                  name=f"pp_{j}_{dtp}")
                    for c in range(2):
                        dt_ = 2 * dtp + c
                        for h in range(HPC):
                            nc.tensor.matmul(
                                pp[:, c, :],
                                wproj_sb[:, h, dt_ * P:(dt_ + 1) * P],
                                o_sb[:, h, :],
                                start=(h == 0), stop=(h == HPC - 1))
                    po = p2sb.tile([P, 2, QB], BF16, tag="po", bufs=4,
                                   name=f"po_{j}_{dtp}")
                    if dtp % 2 == 0:
                        nc.scalar.copy(po[:], pp[:])
                    else:
                        nc.vector.tensor_copy(po[:], pp[:])
                    for c in range(2):
                        dt_ = 2 * dtp + c
                        nc.sync.dma_start(
                            out=out[dt_ * P:(dt_ + 1) * P,
                                    j * QB:(j + 1) * QB],
                            in_=po[:, c, :])

                pending = None          # (j, o_raw, acc) awaiting normalize
                for j in range(NJ):
                    nkt = 4 * j + 4
                    o_ps = [ops_.tile([P, QB], F32, tag="ops",
                                      name=f"ops_{j}_{h}")
                            for h in range(HPC)]
                    acc = p2sb.tile([P, HPC, QB], BF16, tag="acc", bufs=2,
                                    name=f"acc_{j}")

                    exps = {}

                    def lo_of(kt, j=j):
                        return P * (kt - 4 * j) if kt >= 4 * j else 0

                    def s_step(kt, j=j):
                        """score matmuls (both heads into one 2-bank psum)
                        + a single paired exp + triangle mask."""
                        lo = lo_of(kt)
                        sp = big.tile([P, HPC, QB], F32, tag="big",
                                      name=f"sp_{j}_{kt}")
                        for h in range(HPC):
                            nc.tensor.matmul(
                                sp[:, h, lo:QB],
                                kT_[:, h, kt * P:(kt + 1) * P],
                                qT[:, h, j * QB + lo:(j + 1) * QB],
                                start=True, stop=True)
                        e = p2exp.tile([P, HPC, QB], BF16, tag="exp",
                                       name=f"exp_{j}_{kt}")
                        nc.scalar.activation(e[:, :, lo:QB], sp[:, :, lo:QB],
                                             EXP)
                        if kt >= 4 * j:   # diagonal: mask the triangle block
                            nc.vector.tensor_mul(
                                e[:, :, lo:lo + P], e[:, :, lo:lo + P],
                                tri_bf[:, None, :].broadcast_to([P, HPC, P]))
                        exps[kt] = e

                    s_step(0)
                    s_step(1)
                    if j == 0:
                        # deferred last-two-tile transposes, covered by the
                        # first scores (their outputs are needed only by
                        # much later q-blocks)
                        for i in range(2):
                            tt_l = NT - 2 + i
                            for g, dst in ((0, qT), (1, kT_)):
                                tp = big.tile([P, 2, P], BF16, tag="big",
                                              name=f"tp_late{i}_{g}")
                                for c in range(2):
                                    nc.tensor.transpose(
                                        tp[:, c, :],
                                        rp_late[i][:, 2 * g + c, :],
                                        ident[:])
                                nc.scalar.copy(
                                    dst[:, :, tt_l * P:(tt_l + 1) * P],
                                    tp[:])
                    proj_state = None   # [jp, o_sb, next_dtp]
                    for kt in range(nkt):
                        # previous j's normalize + c_proj ride behind our
                        # prologue: their dependencies are off the PE path
                        if kt == 1 and pending is not None:
                            proj_state = [pending[0],
                                          emit_norm(*pending), 0]
                            pending = None
                        if kt + 2 < nkt:
                            s_step(kt + 2)
                        # c_proj pairs paced across the block so the psum
                        # ring and eviction engines are never bursted
                        if kt >= 4 and proj_state is not None:
                            pairs_left = 8 - proj_state[2]
                            steps_left = nkt - kt
                            n_now = -(-pairs_left // steps_left)
                            for _ in range(n_now):
                                emit_proj_pair(proj_state[0], proj_state[1],
                                               proj_state[2])
                                proj_state[2] += 1
                            if proj_state[2] >= 8:
                                proj_state = None
                        e = exps.pop(kt)
                        lo = lo_of(kt)
                        for h in range(HPC):
                            nc.tensor.matmul(o_ps[h][:, lo:QB],
                                             v_sb[:, h, kt, :],
                                             e[:, h, lo:QB],
                                             start=(kt == 0),
                                             stop=(kt == nkt - 1))
                        # denominator accumulation (both heads in one op)
                        if kt == 0:
                            nc.vector.tensor_scalar(
                                out=acc[:], in0=e[:],
                                scalar1=1.0, scalar2=None, op0=MULT)
                        else:
                            nc.vector.tensor_add(
                                acc[:, :, lo:QB], acc[:, :, lo:QB],
                                e[:, :, lo:QB])
                    # leftover proj pairs (small j windows)
                    if proj_state is not None:
                        while proj_state[2] < 8:
                            emit_proj_pair(proj_state[0], proj_state[1],
                                           proj_state[2])
                            proj_state[2] += 1
                        proj_state = None

                    # evict raw o immediately (ACT cast) so the PSUM banks
                    # free up for the next j's AV accumulation
                    o_raw = p2sb.tile([P, HPC, QB], BF16, tag="oraw", bufs=2,
                                      name=f"oraw_{j}")
                    for h in range(HPC):
                        nc.scalar.copy(o_raw[:, h, :], o_ps[h][:])

                    if j == NJ - 1:
                        o_sb = emit_norm(j, o_raw, acc)
                        for dtp in range(8):
                            emit_proj_pair(j, o_sb, dtp)
                    else:
                        pending = (j, o_raw, acc)
                del rp_late

    nc.compile()
    return nc


_NC = None


def _get_nc():
    global _NC
    if _NC is None:
        _NC = _build()
    return _NC


def _bf16():
    import ml_dtypes
    return ml_dtypes.bfloat16


def _host_inputs(x, vi, Wq, Wk, Wv, Wproj, lamb):
    """Build the per-core input maps (all numpy float32)."""
    x = np.asarray(x, dtype=np.float32).reshape(T, D)
    vi = np.asarray(vi, dtype=np.float32).reshape(T, NH, HD)
    Wq = np.asarray(Wq, dtype=np.float32)
    Wk = np.asarray(Wk, dtype=np.float32)
    Wv = np.asarray(Wv, dtype=np.float32)
    Wproj = np.asarray(Wproj, dtype=np.float32)
    lam = float(np.asarray(lamb))

    # x^T tiled: xt[tt, p, kt, f] = x[tt*P+f, kt*P+p]
    xt = np.ascontiguousarray(
        x.reshape(NT, P, KT, P).transpose(0, 3, 2, 1)).astype(_bf16())

    # rope tables
    inv_freq = (1.0 / 10000.0) ** (np.arange(0, HD, 2, dtype=np.float32) / HD)
    tpos = np.arange(T, dtype=np.float32)
    freqs = np.outer(tpos, inv_freq).astype(np.float32)      # [T, 64]
    cs_full = np.concatenate([np.cos(freqs), np.sin(freqs)], axis=1)  # [T,128]
    cs_t = np.ascontiguousarray(
        cs_full.reshape(NT, P, HD).transpose(1, 0, 2))       # [P, NT, HD]

    bf16 = _bf16()
    # causal triangle mask for the diagonal 128x128 blocks of S^T[k, q]:
    # valid iff q-offset >= k-partition
    tri = (np.arange(P)[:, None] <= np.arange(P)[None, :]).astype(bf16)

    in_maps = []
    for core in range(NCORES):
        r0 = core * HPC * HD
        wq_c = Wq[r0:r0 + HPC * HD]
        wk_c = Wk[r0:r0 + HPC * HD]
        wv_c = Wv[r0:r0 + HPC * HD] * (1.0 - lam)
        w_all = np.concatenate(
            [wq_c[0:HD], wq_c[HD:2 * HD],
             wk_c[0:HD], wk_c[HD:2 * HD],
             wv_c[0:HD], wv_c[HD:2 * HD]], axis=0)           # [768, D]
        # wall[p, kt, m] = w_all[m, kt*P+p]  (W_all^T tiled)
        wall_c = np.ascontiguousarray(
            w_all.reshape(6 * HD, KT, P).transpose(2, 1, 0)).astype(_bf16())
        # wproj[p, ct, m] = Wproj[m, r0 + ct*P + p]
        wp = Wproj[:, r0:r0 + HPC * HD]                       # [D, 256]
        wproj_c = np.ascontiguousarray(
            wp.reshape(D, HPC, P).transpose(2, 1, 0)).astype(bf16)
        # vilam[tt, p, c] = lam * vi[tt*P+p, head, hd]
        vl = (lam * vi[:, HPC * core:HPC * (core + 1), :]).reshape(
            NT, P, HPC * HD)
        in_maps.append({
            "xt": xt,
            "wall": wall_c,
            "wproj": wproj_c,
            "vilam": np.ascontiguousarray(vl),
            "cs": cs_t,
            "tri": tri,
            "identr": np.eye(P, dtype=np.float32).astype(bf16),
        })
    return in_maps


def kernel(x, vi, Wq, Wk, Wv, Wproj, lamb, _trace=False, _trace_kwargs=None):
    nc = _get_nc()
    in_maps = _host_inputs(x, vi, Wq, Wk, Wv, Wproj, lamb)
    res = run_bass_kernel_spmd(nc, in_maps, list(range(NCORES)),
                               trace=_trace, **(_trace_kwargs or {}))
    acc = np.zeros((D, T), dtype=np.float64)
    for core in range(NCORES):
        acc += np.asarray(res.results[core]["out"]).astype(np.float64)
    y = acc.T.astype(np.float32).reshape(1, 1, T, D)
    if _trace:
        return y, res
    return y


# revision 19
# speedup vs baseline: 1.0221x; 1.0221x over previous
"""Head-sharded causal self-attention (QK-RMSNorm + RoPE + value-residual mix)
for 8 Trainium2 NeuronCores.

Sharding: 16 heads -> 2 heads per core (tensor parallel). Each core computes
its heads' QKV projections, attention, and a partial c_proj output
[D, T] (transposed, bf16); the host sums the 8 partials (the c_proj
all-reduce).

Layout strategy (per core):
 - QKV matmul produces q,k,v in natural [T, hd] tiles (lhsT = x^T tiles,
   moving = W_all^T), so RMS-norm + RoPE run with T on partitions.
 - q,k are PE-transposed to [hd, T] for the score matmuls.
 - Scores are computed transposed: S^T[T_k, T_q] = k^T_tile.T @ q^T. The
   exp(S^T) tiles (bf16) then directly serve as the moving operand of the
   o^T = v.T @ expS accumulation, so no attention-weight transpose is
   ever needed.
 - Softmax denominator: exp tiles are accumulated per (head, q-block) with
   a chain of bf16 adds (DVE, with every 4th on GpSimd), then ONE
   [1 x QB] ones-matmul per (head, q-block) partition-sums the
   accumulator.  This removes the per-k-tile [128,1,512] PE matmuls the
   previous version spent ~70us on.
 - Causal mask: diagonal-band k-tiles get their score/exp/AV work NARROWED
   to the valid column range; only the [128,128] triangle block needs a
   mask multiply (GpSimd, 0/1 bf16 triangle).
 - o^T[hd, T_q] (normalized, bf16) feeds c_proj: partial^T = Wp^T.T @ o^T.
Matmuls run in float32r for x/w/q/k (accuracy) and bf16 for exp/v/proj
moving operands (1 cycle/row either way; bf16 halves SBUF/DMA traffic).
"""

import numpy as np

import concourse.bacc as bacc
import concourse.mybir as mybir
import concourse.tile as tile
from concourse.bass_utils import run_bass_kernel_spmd

P = 128
T = 4096
D = 2048
HD = 128
NH = 16
HPC = 2            # heads per core
NCORES = 8
NT = T // P        # 32 t-tiles
KT = D // P        # 16 contraction tiles for the projections
NJ = 8             # q-blocks
QB = 512           # q-block width
EPS = 1.1920929e-07

F32 = mybir.dt.float32
F32R = mybir.dt.float32r
BF16 = mybir.dt.bfloat16
MULT = mybir.AluOpType.mult
EXP = mybir.ActivationFunctionType.Exp


def _build():
    nc = bacc.Bacc("TRN2", target_bir_lowering=False, debug=False,
                   enable_asserts=False, num_devices=NCORES)

    # ---- DRAM parameters (host pre-tiled layouts) ----
    xt = nc.dram_tensor("xt", [NT, P, KT, P], BF16, kind="ExternalInput").ap()
    wall = nc.dram_tensor("wall", [P, KT, 6 * HD], BF16, kind="ExternalInput").ap()
    wproj = nc.dram_tensor("wproj", [P, HPC, D], BF16, kind="ExternalInput").ap()
    vilam = nc.dram_tensor("vilam", [NT, P, HPC * HD], F32, kind="ExternalInput").ap()
    cs = nc.dram_tensor("cs", [P, NT, HD], F32, kind="ExternalInput").ap()
    tri = nc.dram_tensor("tri", [P, P], BF16, kind="ExternalInput").ap()
    identr = nc.dram_tensor("identr", [P, P], BF16, kind="ExternalInput").ap()
    out = nc.dram_tensor("out", [D, T], BF16, kind="ExternalOutput").ap()

    with tile.TileContext(nc) as tc:
        with tc.tile_pool(name="persist", bufs=1) as persist:
            # tensors that live for the whole kernel
            qT = persist.tile([P, HPC, T], BF16)        # q^T per head
            kT_ = persist.tile([P, HPC, T], BF16)       # k^T per head
            v_sb = persist.tile([P, HPC, NT, HD], BF16)  # v natural per head
            cs_sbc = [persist.tile([P, 8, HD], F32, name=f"cs_sb{c}")
                      for c in range(4)]
            ident = persist.tile([P, P], BF16)
            tri_bf = persist.tile([P, P], BF16)
            wproj_sb = persist.tile([P, HPC, D], BF16)
            ones_bf = persist.tile([P, 1], BF16)
            eps_q = persist.tile([P, 1], F32)
            eps_k = persist.tile([P, 1], F32)
            nc.gpsimd.memset(eps_q[:], float(P) * EPS)
            nc.gpsimd.memset(eps_k[:], EPS)
            nc.gpsimd.memset(ones_bf[:], 1.0)
            # warm up the gpsimd partition_broadcast ucode path early (its
            # first invocation pays a ~6-7us IRAM load); broadcast is the
            # ONLY op run on GpSimd -- anything else thrashes the Q7 ucode
            # library at ~6us per switch
            warm = persist.tile([P, 8], F32)
            nc.gpsimd.partition_broadcast(warm[:], eps_q[0:1, 0:1]
                                          .broadcast_to([1, 8]))
            # rope outputs of the last two t-tiles persist: their transposes
            # are deferred into phase-2's start so the end of phase 1 does
            # not expose the full ACT->DVE chain latency on the PE queue
            rp_late = [persist.tile([P, 4, P], BF16, name=f"rp_late{i}")
                       for i in range(3)]

            # ---------------- Phase 1: QKV + norm + rope + transposes ----
            with tc.tile_pool(name="p1w", bufs=1) as p1w, \
                 tc.tile_pool(name="p1sb", bufs=4) as p1sb, \
                 tc.tile_pool(name="p1sc", bufs=3) as p1sc, \
                 tc.tile_pool(name="p1ps", bufs=5, space="PSUM") as p1ps, \
                 tc.tile_pool(name="p1tp", bufs=3, space="PSUM") as p1tp:
                # per-(kt,half) weight tiles: fine-grained deps let the first
                # matmuls start as soon as their own chunk has landed
                wall_sb = [[p1w.tile([P, 384], BF16, name=f"wall_sb{kt}_{hf}")
                            for hf in range(2)] for kt in range(KT)]
                # the very first deps: wall lo[0] + first x chunk, issued
                # before anything else so the first matmul starts early
                nc.sync.dma_start(out=wall_sb[0][0][:], in_=wall[:, 0, 0:384])
                x_t0c = [p1w.tile([P, 4, P], BF16, name=f"x_t0c{c}")
                         for c in range(4)]
                nc.sync.dma_start(out=x_t0c[0][:], in_=xt[0, :, 0:4, :])
                nc.sync.dma_start(out=wall_sb[1][0][:], in_=wall[:, 1, 0:384])
                nc.sync.dma_start(out=x_t0c[1][:], in_=xt[0, :, 4:8, :])
                nc.sync.dma_start(out=wall_sb[2][0][:], in_=wall[:, 2, 0:384])
                nc.sync.dma_start(out=x_t0c[2][:], in_=xt[0, :, 8:12, :])
                nc.sync.dma_start(out=wall_sb[3][0][:], in_=wall[:, 3, 0:384])
                nc.sync.dma_start(out=x_t0c[3][:], in_=xt[0, :, 12:16, :])
                nc.sync.dma_start(out=ident[:], in_=identr[:])
                for kt in range(4, KT):
                    nc.sync.dma_start(out=wall_sb[kt][0][:],
                                      in_=wall[:, kt, 0:384])
                vi0 = p1sb.tile([P, HPC * HD], F32, tag="vi", name="vi0")
                nc.sync.dma_start(out=vi0[:], in_=vilam[0])
                for kt in range(KT):
                    nc.sync.dma_start(out=wall_sb[kt][1][:],
                                      in_=wall[:, kt, 384:768])
                x1 = p1sb.tile([P, KT, P], BF16, tag="x", bufs=3, name="x1")
                nc.sync.dma_start(out=x1[:], in_=xt[1])
                nc.sync.dma_start(out=cs_sbc[0][:], in_=cs[:, 0:8, :])

                for tt in range(NT):
                    if 1 <= tt <= 3:
                        nc.sync.dma_start(out=cs_sbc[tt][:],
                                          in_=cs[:, 8 * tt:8 * (tt + 1), :])
                    if tt == 2:
                        nc.sync.dma_start(out=tri_bf[:], in_=tri[:])
                        nc.sync.dma_start(out=wproj_sb[:], in_=wproj[:])
                    if tt == 0:
                        def xop(kt):
                            return x_t0c[kt // 4][:, kt % 4, :]
                    elif tt == 1:
                        def xop(kt, x_t=x1):
                            return x_t[:, kt, :]
                    else:
                        x_t = p1sb.tile([P, KT, P], BF16, tag="x", bufs=3)
                        nc.sync.dma_start(out=x_t[:], in_=xt[tt])

                        def xop(kt, x_t=x_t):
                            return x_t[:, kt, :]
                    if tt == 0:
                        vi_t = vi0
                    else:
                        vi_t = p1sb.tile([P, HPC * HD], F32, tag="vi")
                        nc.sync.dma_start(out=vi_t[:], in_=vilam[tt])
                    halves = []
                    for half in range(2):
                        ps = p1ps.tile([P, 384], F32, tag="qkvps")
                        for kt in range(KT):
                            nc.tensor.matmul(
                                ps[:],
                                xop(kt),
                                wall_sb[kt][half][:],
                                start=(kt == 0),
                                stop=(kt == KT - 1),
                            )
                        halves.append(ps)

                    # --- evict q,k into natural tile + per-row sum-squares
                    qk_nat = p1sb.tile([P, 4, P], F32, tag="qknat")
                    ssq = p1sc.tile([P, 4], F32, tag="ssq")
                    sqs = p1sb.tile([P, P], F32, tag="sqscratch")
                    nc.scalar.copy(qk_nat[:, 0:3, :], halves[0][:, 0:384])
                    nc.scalar.copy(qk_nat[:, 3, :], halves[1][:, 0:P])
                    # squares read the SBUF copy, not PSUM: the qkv psum
                    # then frees right after the copies + v-adds
                    for i in range(4):          # q0 q1 k0 k1
                        nc.scalar.activation(
                            sqs[:], qk_nat[:, i, :],
                            mybir.ActivationFunctionType.Square,
                            accum_out=ssq[:, i:i + 1])
                    # --- v: psum + lam*vi -> bf16 natural tile
                    for h in range(HPC):
                        nc.vector.tensor_add(
                            v_sb[:, h, tt, :],
                            halves[1][:, P + h * P:P + (h + 1) * P],
                            vi_t[:, h * P:(h + 1) * P])

                    # --- rms scales: q -> 1/sqrt(ssq+128eps) (incl 1/sqrt(hd));
                    #     k rows also need a sqrt(128) factor
                    sca = p1sc.tile([P, 4], F32, tag="sca")
                    rsc = p1sc.tile([P, 4], F32, tag="rsc")
                    nc.scalar.activation(sca[:], ssq[:],
                                         mybir.ActivationFunctionType.Sqrt,
                                         bias=eps_q[:], scale=1.0)
                    nc.vector.reciprocal(rsc[:], sca[:])

                    # --- scale q and k by their rms scales
                    for i in range(4):
                        nc.vector.tensor_scalar(
                            out=qk_nat[:, i, :], in0=qk_nat[:, i, :],
                            scalar1=rsc[:, i:i + 1],
                            scalar2=(1.0 if i < 2 else float(np.sqrt(P))),
                            op0=MULT, op1=MULT)

                    # --- rope on all 4 tensors at once (f32r out: final
                    # values ahead of the f32r transpose + score matmuls)
                    if tt >= NT - 3:
                        rp = rp_late[tt - (NT - 3)]
                    else:
                        rp = p1sb.tile([P, 4, P], BF16, tag="rope", bufs=4)
                    tmp = p1sb.tile([P, 4, 64], F32, tag="ropetmp")
                    x1_ = qk_nat[:, :, 0:64]
                    x2 = qk_nat[:, :, 64:128]
                    cst = cs_sbc[tt // 8]
                    cb = cst[:, tt % 8, None, 0:64].broadcast_to([P, 4, 64])
                    sb = cst[:, tt % 8, None, 64:128].broadcast_to([P, 4, 64])
                    nc.vector.tensor_mul(rp[:, :, 0:64], x1_, cb)
                    nc.vector.tensor_mul(tmp[:], x2, sb)
                    nc.vector.tensor_add(rp[:, :, 0:64], rp[:, :, 0:64], tmp[:])
                    nc.vector.tensor_mul(rp[:, :, 64:128], x2, cb)
                    nc.vector.tensor_mul(tmp[:], x1_, sb)
                    nc.vector.tensor_sub(rp[:, :, 64:128], rp[:, :, 64:128], tmp[:])

                    # --- transpose q,k of the PREVIOUS tile (deferred
                    # one iteration: by now its rope is long done, so these
                    # never park in the 4-deep PE wait queue); the last
                    # three tiles are deferred into phase 2
                    if 1 <= tt <= NT - 3:
                        tl, rpl = tt - 1, rp_prev
                        for g, dst in ((0, qT), (1, kT_)):
                            tp = p1tp.tile([P, 2, P], BF16, tag="tp", bufs=3)
                            for i in range(2):
                                nc.tensor.transpose(tp[:, i, :],
                                                    rpl[:, 2 * g + i, :],
                                                    ident[:])
                            nc.scalar.copy(dst[:, :, tl * P:(tl + 1) * P],
                                           tp[:])
                    rp_prev = rp

            # ---------------- Phase 2+3: attention + c_proj ----
            with tc.tile_pool(name="p2exp", bufs=8) as p2exp, \
                 tc.tile_pool(name="p2sb", bufs=4) as p2sb, \
                 tc.tile_pool(name="p2sc", bufs=4) as p2sc, \
                 tc.tile_pool(name="big", bufs=3, space="PSUM") as big, \
                 tc.tile_pool(name="ops", bufs=2, space="PSUM") as ops_:

                def emit_norm(j, o_raw, acc):
                    """softmax-normalize j's raw o into o_sb. o_ps was
                    already released by the o_raw cast, so this chain is
                    latency-tolerant."""
                    o_sb = p2sb.tile([P, HPC, QB], BF16, tag="osb", bufs=3,
                                     name=f"osb_{j}")
                    for h in range(HPC):
                        den = big.tile([1, QB], F32, tag="big",
                                       name=f"den_{j}_{h}")
                        nc.tensor.matmul(den[:], ones_bf[:], acc[:, h, :],
                                         start=True, stop=True)
                        rden = p2sc.tile([1, QB], F32, tag="rden")
                        nc.vector.reciprocal_approx_fast(rden[:], den[:])
                        bc = p2sb.tile([P, QB], F32, tag="bc")
                        nc.gpsimd.partition_broadcast(bc[:], rden[:])
                        nc.vector.tensor_mul(o_sb[:, h, :], o_raw[:, h, :],
                                             bc[:])
                    return o_sb

                def emit_proj_pair(j, o_sb, dtp):
                    """c_proj for two adjacent 128-row output blocks."""
                    pp = big.tile([P, 2, QB], F32, tag="big",
                                  name=f"pp_{j}_{dtp}")
                    for c in range(2):
                        dt_ = 2 * dtp + c
                        for h in range(HPC):
                            nc.tensor.matmul(
                                pp[:, c, :],
                                wproj_sb[:, h, dt_ * P:(dt_ + 1) * P],
                                o_sb[:, h, :],
                                start=(h == 0), stop=(h == HPC - 1))
                    po = p2sb.tile([P, 2, QB], BF16, tag="po", bufs=4,
                                   name=f"po_{j}_{dtp}")
                    if dtp % 2 == 0:
                        nc.scalar.copy(po[:], pp[:])
                    else:
                        nc.vector.tensor_copy(po[:], pp[:])
                    for c in range(2):
                        dt_ = 2 * dtp + c
                        nc.sync.dma_start(
                            out=out[dt_ * P:(dt_ + 1) * P,
                                    j * QB:(j + 1) * QB],
                            in_=po[:, c, :])

                pending = None          # (j, o_raw, acc) awaiting normalize
                for j in range(NJ):
                    nkt = 4 * j + 4
                    o_ps = [ops_.tile([P, QB], F32, tag="ops",
                                      name=f"ops_{j}_{h}")
                            for h in range(HPC)]
                    acc = p2sb.tile([P, HPC, QB], BF16, tag="acc", bufs=2,
                                    name=f"acc_{j}")

                    exps = {}

                    def lo_of(kt, j=j):
                        return P * (kt - 4 * j) if kt >= 4 * j else 0

                    def s_step(kt, j=j):
                        """score matmuls (both heads into one 2-bank psum)
                        + a single paired exp + triangle mask."""
                        lo = lo_of(kt)
                        sp = big.tile([P, HPC, QB], F32, tag="big",
                                      name=f"sp_{j}_{kt}")
                        for h in range(HPC):
                            nc.tensor.matmul(
                                sp[:, h, lo:QB],
                                kT_[:, h, kt * P:(kt + 1) * P],
                                qT[:, h, j * QB + lo:(j + 1) * QB],
                                start=True, stop=True)
                        e = p2exp.tile([P, HPC, QB], BF16, tag="exp",
                                       name=f"exp_{j}_{kt}")
                        nc.scalar.activation(e[:, :, lo:QB], sp[:, :, lo:QB],
                                             EXP)
                        if kt >= 4 * j:   # diagonal: mask the triangle block
                            nc.vector.tensor_mul(
                                e[:, :, lo:lo + P], e[:, :, lo:lo + P],
                                tri_bf[:, None, :].broadcast_to([P, HPC, P]))
                        exps[kt] = e

                    s_step(0)
                    s_step(1)
                    if j == 0:
                        # deferred last-three-tile transposes, covered by the
                        # first scores (their outputs are needed only by
                        # much later q-blocks)
                        for i in range(3):
                            tt_l = NT - 3 + i
                            for g, dst in ((0, qT), (1, kT_)):
                                tp = big.tile([P, 2, P], BF16, tag="big",
                                              name=f"tp_late{i}_{g}")
                                for c in range(2):
                                    nc.tensor.transpose(
                                        tp[:, c, :],
                                        rp_late[i][:, 2 * g + c, :],
                                        ident[:])
                                nc.scalar.copy(
                                    dst[:, :, tt_l * P:(tt_l + 1) * P],
                                    tp[:])
                    proj_state = None   # [jp, o_sb, next_dtp]
                    for kt in range(nkt):
                        # previous j's normalize + c_proj ride behind our
                        # prologue: their dependencies are off the PE path
                        if kt == 1 and pending is not None:
                            proj_state = [pending[0],
                                          emit_norm(*pending), 0]
                            pending = None
                        if kt + 2 < nkt:
                            s_step(kt + 2)
                        # c_proj pairs paced across the block so the psum
                        # ring and eviction engines are never bursted
                        if kt >= 4 and proj_state is not None:
                            pairs_left = 8 - proj_state[2]
                            steps_left = nkt - kt
                            n_now = -(-pairs_left // steps_left)
                            for _ in range(n_now):
                                emit_proj_pair(proj_state[0], proj_state[1],
                                               proj_state[2])
                                proj_state[2] += 1
                            if proj_state[2] >= 8:
                                proj_state = None
                        e = exps.pop(kt)
                        lo = lo_of(kt)
                        for h in range(HPC):
                            nc.tensor.matmul(o_ps[h][:, lo:QB],
                                             v_sb[:, h, kt, :],
                                             e[:, h, lo:QB],
                                             start=(kt == 0),
                                             stop=(kt == nkt - 1))
                        # denominator accumulation (both heads in one op)
                        if kt == 0:
                            nc.vector.tensor_scalar(
                                out=acc[:], in0=e[:],
                                scalar1=1.0, scalar2=None, op0=MULT)
                        else:
                            nc.vector.tensor_add(
                                acc[:, :, lo:QB], acc[:, :, lo:QB],
                                e[:, :, lo:QB])
                    # leftover proj pairs (small j windows)
                    if proj_state is not None:
                        while proj_state[2] < 8:
                            emit_proj_pair(proj_state[0], proj_state[1],
                                           proj_state[2])
                            proj_state[2] += 1
                        proj_state = None

                    # evict raw o immediately (ACT cast) so the PSUM banks
                    # free up for the next j's AV accumulation
                    o_raw = p2sb.tile([P, HPC, QB], BF16, tag="oraw", bufs=2,
                                      name=f"oraw_{j}")
                    for h in range(HPC):
                        nc.scalar.copy(o_raw[:, h, :], o_ps[h][:])

                    if j == NJ - 1:
                        o_sb = emit_norm(j, o_raw, acc)
                        for dtp in range(8):
                            emit_proj_pair(j, o_sb, dtp)
                    else:
                        pending = (j, o_raw, acc)
                del rp_late

    nc.compile()
    return nc


_NC = None


def _get_nc():
    global _NC
    if _NC is None:
        _NC = _build()
    return _NC


def _bf16():
    import ml_dtypes
    return ml_dtypes.bfloat16


def _host_inputs(x, vi, Wq, Wk, Wv, Wproj, lamb):
    """Build the per-core input maps (all numpy float32)."""
    x = np.asarray(x, dtype=np.float32).reshape(T, D)
    vi = np.asarray(vi, dtype=np.float32).reshape(T, NH, HD)
    Wq = np.asarray(Wq, dtype=np.float32)
    Wk = np.asarray(Wk, dtype=np.float32)
    Wv = np.asarray(Wv, dtype=np.float32)
    Wproj = np.asarray(Wproj, dtype=np.float32)
    lam = float(np.asarray(lamb))

    # x^T tiled: xt[tt, p, kt, f] = x[tt*P+f, kt*P+p]
    xt = np.ascontiguousarray(
        x.reshape(NT, P, KT, P).transpose(0, 3, 2, 1)).astype(_bf16())

    # rope tables
    inv_freq = (1.0 / 10000.0) ** (np.arange(0, HD, 2, dtype=np.float32) / HD)
    tpos = np.arange(T, dtype=np.float32)
    freqs = np.outer(tpos, inv_freq).astype(np.float32)      # [T, 64]
    cs_full = np.concatenate([np.cos(freqs), np.sin(freqs)], axis=1)  # [T,128]
    cs_t = np.ascontiguousarray(
        cs_full.reshape(NT, P, HD).transpose(1, 0, 2))       # [P, NT, HD]

    bf16 = _bf16()
    # causal triangle mask for the diagonal 128x128 blocks of S^T[k, q]:
    # valid iff q-offset >= k-partition
    tri = (np.arange(P)[:, None] <= np.arange(P)[None, :]).astype(bf16)

    in_maps = []
    for core in range(NCORES):
        r0 = core * HPC * HD
        wq_c = Wq[r0:r0 + HPC * HD]
        wk_c = Wk[r0:r0 + HPC * HD]
        wv_c = Wv[r0:r0 + HPC * HD] * (1.0 - lam)
        w_all = np.concatenate(
            [wq_c[0:HD], wq_c[HD:2 * HD],
             wk_c[0:HD], wk_c[HD:2 * HD],
             wv_c[0:HD], wv_c[HD:2 * HD]], axis=0)           # [768, D]
        # wall[p, kt, m] = w_all[m, kt*P+p]  (W_all^T tiled)
        wall_c = np.ascontiguousarray(
            w_all.reshape(6 * HD, KT, P).transpose(2, 1, 0)).astype(_bf16())
        # wproj[p, ct, m] = Wproj[m, r0 + ct*P + p]
        wp = Wproj[:, r0:r0 + HPC * HD]                       # [D, 256]
        wproj_c = np.ascontiguousarray(
            wp.reshape(D, HPC, P).transpose(2, 1, 0)).astype(bf16)
        # vilam[tt, p, c] = lam * vi[tt*P+p, head, hd]
        vl = (lam * vi[:, HPC * core:HPC * (core + 1), :]).reshape(
            NT, P, HPC * HD)
        in_maps.append({
            "xt": xt,
            "wall": wall_c,
            "wproj": wproj_c,
            "vilam": np.ascontiguousarray(vl),
            "cs": cs_t,
            "tri": tri,
            "identr": np.eye(P, dtype=np.float32).astype(bf16),
        })
    return in_maps


def kernel(x, vi, Wq, Wk, Wv, Wproj, lamb, _trace=False, _trace_kwargs=None):
    nc = _get_nc()
    in_maps = _host_inputs(x, vi, Wq, Wk, Wv, Wproj, lamb)
    res = run_bass_kernel_spmd(nc, in_maps, list(range(NCORES)),
                               trace=_trace, **(_trace_kwargs or {}))
    acc = np.zeros((D, T), dtype=np.float64)
    for core in range(NCORES):
        acc += np.asarray(res.results[core]["out"]).astype(np.float64)
    y = acc.T.astype(np.float32).reshape(1, 1, T, D)
    if _trace:
        return y, res
    return y


# revision 20
# speedup vs baseline: 1.0512x; 1.0285x over previous
"""Head-sharded causal self-attention (QK-RMSNorm + RoPE + value-residual mix)
for 8 Trainium2 NeuronCores.

Sharding: 16 heads -> 2 heads per core (tensor parallel). Each core computes
its heads' QKV projections, attention, and a partial c_proj output
[D, T] (transposed, bf16); the host sums the 8 partials (the c_proj
all-reduce).

Layout strategy (per core):
 - QKV matmul produces q,k,v in natural [T, hd] tiles (lhsT = x^T tiles,
   moving = W_all^T), so RMS-norm + RoPE run with T on partitions.
 - q,k are PE-transposed to [hd, T] for the score matmuls.
 - Scores are computed transposed: S^T[T_k, T_q] = k^T_tile.T @ q^T. The
   exp(S^T) tiles (bf16) then directly serve as the moving operand of the
   o^T = v.T @ expS accumulation, so no attention-weight transpose is
   ever needed.
 - Softmax denominator: exp tiles are accumulated per (head, q-block) with
   a chain of bf16 adds (DVE, with every 4th on GpSimd), then ONE
   [1 x QB] ones-matmul per (head, q-block) partition-sums the
   accumulator.  This removes the per-k-tile [128,1,512] PE matmuls the
   previous version spent ~70us on.
 - Causal mask: diagonal-band k-tiles get their score/exp/AV work NARROWED
   to the valid column range; only the [128,128] triangle block needs a
   mask multiply (GpSimd, 0/1 bf16 triangle).
 - o^T[hd, T_q] (normalized, bf16) feeds c_proj: partial^T = Wp^T.T @ o^T.
Matmuls run in float32r for x/w/q/k (accuracy) and bf16 for exp/v/proj
moving operands (1 cycle/row either way; bf16 halves SBUF/DMA traffic).
"""

import numpy as np

import concourse.bacc as bacc
import concourse.mybir as mybir
import concourse.tile as tile
from concourse.bass_utils import run_bass_kernel_spmd

P = 128
T = 4096
D = 2048
HD = 128
NH = 16
HPC = 2            # heads per core
NCORES = 8
NT = T // P        # 32 t-tiles
KT = D // P        # 16 contraction tiles for the projections
NJ = 8             # q-blocks
QB = 512           # q-block width
EPS = 1.1920929e-07

F32 = mybir.dt.float32
F32R = mybir.dt.float32r
BF16 = mybir.dt.bfloat16
MULT = mybir.AluOpType.mult
EXP = mybir.ActivationFunctionType.Exp


def _build():
    nc = bacc.Bacc("TRN2", target_bir_lowering=False, debug=False,
                   enable_asserts=False, num_devices=NCORES)

    # ---- DRAM parameters (host pre-tiled layouts) ----
    xt = nc.dram_tensor("xt", [NT, P, KT, P], BF16, kind="ExternalInput").ap()
    wall = nc.dram_tensor("wall", [P, KT, 6 * HD], BF16, kind="ExternalInput").ap()
    wproj = nc.dram_tensor("wproj", [P, HPC, D], BF16, kind="ExternalInput").ap()
    vilam = nc.dram_tensor("vilam", [NT, P, HPC * HD], F32, kind="ExternalInput").ap()
    cs = nc.dram_tensor("cs", [P, NT, HD], F32, kind="ExternalInput").ap()
    tri = nc.dram_tensor("tri", [P, P], BF16, kind="ExternalInput").ap()
    identr = nc.dram_tensor("identr", [P, P], BF16, kind="ExternalInput").ap()
    out = nc.dram_tensor("out", [D, T], BF16, kind="ExternalOutput").ap()

    with tile.TileContext(nc) as tc:
        with tc.tile_pool(name="persist", bufs=1) as persist:
            # tensors that live for the whole kernel
            qT = persist.tile([P, HPC, T], BF16)        # q^T per head
            kT_ = persist.tile([P, HPC, T], BF16)       # k^T per head
            v_sb = persist.tile([P, HPC, NT, HD], BF16)  # v natural per head
            cs_sbc = [persist.tile([P, 8, HD], F32, name=f"cs_sb{c}")
                      for c in range(4)]
            ident = persist.tile([P, P], BF16)
            tri_bf = persist.tile([P, P], BF16)
            wproj_sb = persist.tile([P, HPC, D], BF16)
            ones_bf = persist.tile([P, 1], BF16)
            eps_q = persist.tile([P, 1], F32)
            eps_k = persist.tile([P, 1], F32)
            nc.gpsimd.memset(eps_q[:], float(P) * EPS)
            nc.gpsimd.memset(eps_k[:], EPS)
            nc.gpsimd.memset(ones_bf[:], 1.0)
            # warm up the gpsimd partition_broadcast ucode path early (its
            # first invocation pays a ~6-7us IRAM load); broadcast is the
            # ONLY op run on GpSimd -- anything else thrashes the Q7 ucode
            # library at ~6us per switch
            warm = persist.tile([P, 8], F32)
            nc.gpsimd.partition_broadcast(warm[:], eps_q[0:1, 0:1]
                                          .broadcast_to([1, 8]))
            # rope outputs of the last two t-tiles persist: their transposes
            # are deferred into phase-2's start so the end of phase 1 does
            # not expose the full ACT->DVE chain latency on the PE queue
            rp_late = [persist.tile([P, 4, P], BF16, name=f"rp_late{i}")
                       for i in range(3)]

            # ---------------- Phase 1: QKV + norm + rope + transposes ----
            with tc.tile_pool(name="p1w", bufs=1) as p1w, \
                 tc.tile_pool(name="p1sb", bufs=4) as p1sb, \
                 tc.tile_pool(name="p1sc", bufs=3) as p1sc, \
                 tc.tile_pool(name="p1ps", bufs=5, space="PSUM") as p1ps, \
                 tc.tile_pool(name="p1tp", bufs=3, space="PSUM") as p1tp:
                # per-(kt,half) weight tiles: fine-grained deps let the first
                # matmuls start as soon as their own chunk has landed
                wall_sb = [[p1w.tile([P, 384], BF16, name=f"wall_sb{kt}_{hf}")
                            for hf in range(2)] for kt in range(KT)]
                # the very first deps: wall lo[0] + first x chunk, issued
                # before anything else so the first matmul starts early
                nc.sync.dma_start(out=wall_sb[0][0][:], in_=wall[:, 0, 0:384])
                x_t0c = [p1w.tile([P, 4, P], BF16, name=f"x_t0c{c}")
                         for c in range(4)]
                nc.sync.dma_start(out=x_t0c[0][:], in_=xt[0, :, 0:4, :])
                nc.sync.dma_start(out=wall_sb[1][0][:], in_=wall[:, 1, 0:384])
                nc.sync.dma_start(out=x_t0c[1][:], in_=xt[0, :, 4:8, :])
                nc.sync.dma_start(out=wall_sb[2][0][:], in_=wall[:, 2, 0:384])
                nc.sync.dma_start(out=x_t0c[2][:], in_=xt[0, :, 8:12, :])
                nc.sync.dma_start(out=wall_sb[3][0][:], in_=wall[:, 3, 0:384])
                nc.sync.dma_start(out=x_t0c[3][:], in_=xt[0, :, 12:16, :])
                nc.sync.dma_start(out=ident[:], in_=identr[:])
                # issue order tracks PE need-time (the SP sequencer issues
                # one dma_start per ~565ns, so ordering is load-bearing):
                # lo[4..9], hi[0..3], x1, lo[10..15], hi[4..9], x2,
                # vi0, cs0, hi[10..15], x3
                for kt in range(4, 10):
                    nc.sync.dma_start(out=wall_sb[kt][0][:],
                                      in_=wall[:, kt, 0:384])
                for kt in range(4):
                    nc.sync.dma_start(out=wall_sb[kt][1][:],
                                      in_=wall[:, kt, 384:768])
                x1 = p1sb.tile([P, KT, P], BF16, tag="x", bufs=4, name="x1")
                nc.sync.dma_start(out=x1[:], in_=xt[1])
                for kt in range(10, KT):
                    nc.sync.dma_start(out=wall_sb[kt][0][:],
                                      in_=wall[:, kt, 0:384])
                for kt in range(4, 10):
                    nc.sync.dma_start(out=wall_sb[kt][1][:],
                                      in_=wall[:, kt, 384:768])
                x2 = p1sb.tile([P, KT, P], BF16, tag="x", bufs=4, name="x2")
                nc.sync.dma_start(out=x2[:], in_=xt[2])
                vi0 = p1sb.tile([P, HPC * HD], F32, tag="vi", name="vi0")
                nc.sync.dma_start(out=vi0[:], in_=vilam[0])
                nc.sync.dma_start(out=cs_sbc[0][:], in_=cs[:, 0:8, :])
                for kt in range(10, KT):
                    nc.sync.dma_start(out=wall_sb[kt][1][:],
                                      in_=wall[:, kt, 384:768])
                x3 = p1sb.tile([P, KT, P], BF16, tag="x", bufs=4, name="x3")
                nc.sync.dma_start(out=x3[:], in_=xt[3])
                xpre = {1: x1, 2: x2, 3: x3}

                for tt in range(NT):
                    if 1 <= tt <= 3:
                        nc.sync.dma_start(out=cs_sbc[tt][:],
                                          in_=cs[:, 8 * tt:8 * (tt + 1), :])
                    if tt == 16:
                        nc.sync.dma_start(out=tri_bf[:], in_=tri[:])
                        nc.sync.dma_start(out=wproj_sb[:], in_=wproj[:])
                    if tt == 0:
                        def xop(kt):
                            return x_t0c[kt // 4][:, kt % 4, :]
                    elif tt <= 3:
                        def xop(kt, x_t=xpre[tt]):
                            return x_t[:, kt, :]
                    else:
                        x_t = p1sb.tile([P, KT, P], BF16, tag="x", bufs=4)
                        nc.sync.dma_start(out=x_t[:], in_=xt[tt])

                        def xop(kt, x_t=x_t):
                            return x_t[:, kt, :]
                    if tt == 0:
                        vi_t = vi0
                    else:
                        vi_t = p1sb.tile([P, HPC * HD], F32, tag="vi")
                        nc.sync.dma_start(out=vi_t[:], in_=vilam[tt])
                    halves = []
                    for half in range(2):
                        ps = p1ps.tile([P, 384], F32, tag="qkvps")
                        for kt in range(KT):
                            nc.tensor.matmul(
                                ps[:],
                                xop(kt),
                                wall_sb[kt][half][:],
                                start=(kt == 0),
                                stop=(kt == KT - 1),
                            )
                        halves.append(ps)

                    # --- evict q,k into natural tile + per-row sum-squares
                    qk_nat = p1sb.tile([P, 4, P], F32, tag="qknat")
                    ssq = p1sc.tile([P, 4], F32, tag="ssq")
                    sqs = p1sb.tile([P, P], F32, tag="sqscratch")
                    nc.scalar.copy(qk_nat[:, 0:3, :], halves[0][:, 0:384])
                    nc.scalar.copy(qk_nat[:, 3, :], halves[1][:, 0:P])
                    # squares read the SBUF copy, not PSUM: the qkv psum
                    # then frees right after the copies + v-adds
                    for i in range(4):          # q0 q1 k0 k1
                        nc.scalar.activation(
                            sqs[:], qk_nat[:, i, :],
                            mybir.ActivationFunctionType.Square,
                            accum_out=ssq[:, i:i + 1])
                    # --- v: psum + lam*vi -> bf16 natural tile
                    for h in range(HPC):
                        nc.vector.tensor_add(
                            v_sb[:, h, tt, :],
                            halves[1][:, P + h * P:P + (h + 1) * P],
                            vi_t[:, h * P:(h + 1) * P])

                    # --- rms scales: q -> 1/sqrt(ssq+128eps) (incl 1/sqrt(hd));
                    #     k rows also need a sqrt(128) factor
                    sca = p1sc.tile([P, 4], F32, tag="sca")
                    rsc = p1sc.tile([P, 4], F32, tag="rsc")
                    nc.scalar.activation(sca[:], ssq[:],
                                         mybir.ActivationFunctionType.Sqrt,
                                         bias=eps_q[:], scale=1.0)
                    nc.vector.reciprocal(rsc[:], sca[:])

                    # --- scale q and k by their rms scales
                    for i in range(4):
                        nc.vector.tensor_scalar(
                            out=qk_nat[:, i, :], in0=qk_nat[:, i, :],
                            scalar1=rsc[:, i:i + 1],
                            scalar2=(1.0 if i < 2 else float(np.sqrt(P))),
                            op0=MULT, op1=MULT)

                    # --- rope on all 4 tensors at once (f32r out: final
                    # values ahead of the f32r transpose + score matmuls)
                    if tt >= NT - 3:
                        rp = rp_late[tt - (NT - 3)]
                    else:
                        rp = p1sb.tile([P, 4, P], BF16, tag="rope", bufs=4)
                    tmp = p1sb.tile([P, 4, 64], F32, tag="ropetmp")
                    x1_ = qk_nat[:, :, 0:64]
                    x2 = qk_nat[:, :, 64:128]
                    cst = cs_sbc[tt // 8]
                    cb = cst[:, tt % 8, None, 0:64].broadcast_to([P, 4, 64])
                    sb = cst[:, tt % 8, None, 64:128].broadcast_to([P, 4, 64])
                    nc.vector.tensor_mul(rp[:, :, 0:64], x1_, cb)
                    nc.vector.tensor_mul(tmp[:], x2, sb)
                    nc.vector.tensor_add(rp[:, :, 0:64], rp[:, :, 0:64], tmp[:])
                    nc.vector.tensor_mul(rp[:, :, 64:128], x2, cb)
                    nc.vector.tensor_mul(tmp[:], x1_, sb)
                    nc.vector.tensor_sub(rp[:, :, 64:128], rp[:, :, 64:128], tmp[:])

                    # --- transpose q,k of the PREVIOUS tile (deferred
                    # one iteration: by now its rope is long done, so these
                    # never park in the 4-deep PE wait queue); the last
                    # three tiles are deferred into phase 2
                    if 1 <= tt <= NT - 3:
                        tl, rpl = tt - 1, rp_prev
                        for g, dst in ((0, qT), (1, kT_)):
                            tp = p1tp.tile([P, 2, P], BF16, tag="tp", bufs=3)
                            for i in range(2):
                                nc.tensor.transpose(tp[:, i, :],
                                                    rpl[:, 2 * g + i, :],
                                                    ident[:])
                            nc.scalar.copy(dst[:, :, tl * P:(tl + 1) * P],
                                           tp[:])
                    rp_prev = rp

            # ---------------- Phase 2+3: attention + c_proj ----
            with tc.tile_pool(name="p2exp", bufs=8) as p2exp, \
                 tc.tile_pool(name="p2sb", bufs=4) as p2sb, \
                 tc.tile_pool(name="p2sc", bufs=4) as p2sc, \
                 tc.tile_pool(name="big", bufs=3, space="PSUM") as big, \
                 tc.tile_pool(name="ops", bufs=2, space="PSUM") as ops_:

                def emit_norm(j, o_raw, acc):
                    """softmax-normalize j's raw o into o_sb. o_ps was
                    already released by the o_raw cast, so this chain is
                    latency-tolerant."""
                    o_sb = p2sb.tile([P, HPC, QB], BF16, tag="osb", bufs=3,
                                     name=f"osb_{j}")
                    for h in range(HPC):
                        den = big.tile([1, QB], F32, tag="big",
                                       name=f"den_{j}_{h}")
                        nc.tensor.matmul(den[:], ones_bf[:], acc[:, h, :],
                                         start=True, stop=True)
                        rden = p2sc.tile([1, QB], F32, tag="rden")
                        nc.vector.reciprocal_approx_fast(rden[:], den[:])
                        bc = p2sb.tile([P, QB], F32, tag="bc")
                        nc.gpsimd.partition_broadcast(bc[:], rden[:])
                        nc.vector.tensor_mul(o_sb[:, h, :], o_raw[:, h, :],
                                             bc[:])
                    return o_sb

                def emit_proj_pair(j, o_sb, dtp):
                    """c_proj for two adjacent 128-row output blocks."""
                    pp = big.tile([P, 2, QB], F32, tag="big",
                                  name=f"pp_{j}_{dtp}")
                    for c in range(2):
                        dt_ = 2 * dtp + c
                        for h in range(HPC):
                            nc.tensor.matmul(
                                pp[:, c, :],
                                wproj_sb[:, h, dt_ * P:(dt_ + 1) * P],
                                o_sb[:, h, :],
                                start=(h == 0), stop=(h == HPC - 1))
                    po = p2sb.tile([P, 2, QB], BF16, tag="po", bufs=4,
                                   name=f"po_{j}_{dtp}")
                    if dtp % 2 == 0:
                        nc.scalar.copy(po[:], pp[:])
                    else:
                        nc.vector.tensor_copy(po[:], pp[:])
                    for c in range(2):
                        dt_ = 2 * dtp + c
                        nc.sync.dma_start(
                            out=out[dt_ * P:(dt_ + 1) * P,
                                    j * QB:(j + 1) * QB],
                            in_=po[:, c, :])

                pending = None          # (j, o_raw, acc) awaiting normalize
                for j in range(NJ):
                    nkt = 4 * j + 4
                    o_ps = [ops_.tile([P, QB], F32, tag="ops",
                                      name=f"ops_{j}_{h}")
                            for h in range(HPC)]
                    acc = p2sb.tile([P, HPC, QB], BF16, tag="acc", bufs=2,
                                    name=f"acc_{j}")

                    exps = {}

                    def lo_of(kt, j=j):
                        return P * (kt - 4 * j) if kt >= 4 * j else 0

                    def s_step(kt, j=j):
                        """score matmuls (both heads into one 2-bank psum)
                        + a single paired exp + triangle mask."""
                        lo = lo_of(kt)
                        sp = big.tile([P, HPC, QB], F32, tag="big",
                                      name=f"sp_{j}_{kt}")
                        for h in range(HPC):
                            nc.tensor.matmul(
                                sp[:, h, lo:QB],
                                kT_[:, h, kt * P:(kt + 1) * P],
                                qT[:, h, j * QB + lo:(j + 1) * QB],
                                start=True, stop=True)
                        e = p2exp.tile([P, HPC, QB], BF16, tag="exp",
                                       name=f"exp_{j}_{kt}")
                        nc.scalar.activation(e[:, :, lo:QB], sp[:, :, lo:QB],
                                             EXP)
                        if kt >= 4 * j:   # diagonal: mask the triangle block
                            nc.vector.tensor_mul(
                                e[:, :, lo:lo + P], e[:, :, lo:lo + P],
                                tri_bf[:, None, :].broadcast_to([P, HPC, P]))
                        exps[kt] = e

                    s_step(0)
                    s_step(1)
                    if j == 0:
                        # deferred last-three-tile transposes, covered by the
                        # first scores (their outputs are needed only by
                        # much later q-blocks)
                        for i in range(3):
                            tt_l = NT - 3 + i
                            for g, dst in ((0, qT), (1, kT_)):
                                tp = big.tile([P, 2, P], BF16, tag="big",
                                              name=f"tp_late{i}_{g}")
                                for c in range(2):
                                    nc.tensor.transpose(
                                        tp[:, c, :],
                                        rp_late[i][:, 2 * g + c, :],
                                        ident[:])
                                nc.scalar.copy(
                                    dst[:, :, tt_l * P:(tt_l + 1) * P],
                                    tp[:])
                    proj_state = None   # [jp, o_sb, next_dtp]
                    for kt in range(nkt):
                        # previous j's normalize + c_proj ride behind our
                        # prologue: their dependencies are off the PE path
                        if kt == 1 and pending is not None:
                            proj_state = [pending[0],
                                          emit_norm(*pending), 0]
                            pending = None
                        if kt + 2 < nkt:
                            s_step(kt + 2)
                        # c_proj pairs paced across the block so the psum
                        # ring and eviction engines are never bursted
                        if kt >= 4 and proj_state is not None:
                            pairs_left = 8 - proj_state[2]
                            steps_left = nkt - kt
                            n_now = -(-pairs_left // steps_left)
                            for _ in range(n_now):
                                emit_proj_pair(proj_state[0], proj_state[1],
                                               proj_state[2])
                                proj_state[2] += 1
                            if proj_state[2] >= 8:
                                proj_state = None
                        e = exps.pop(kt)
                        lo = lo_of(kt)
                        for h in range(HPC):
                            nc.tensor.matmul(o_ps[h][:, lo:QB],
                                             v_sb[:, h, kt, :],
                                             e[:, h, lo:QB],
                                             start=(kt == 0),
                                             stop=(kt == nkt - 1))
                        # denominator accumulation (both heads in one op)
                        if kt == 0:
                            nc.vector.tensor_scalar(
                                out=acc[:], in0=e[:],
                                scalar1=1.0, scalar2=None, op0=MULT)
                        else:
                            nc.vector.tensor_add(
                                acc[:, :, lo:QB], acc[:, :, lo:QB],
                                e[:, :, lo:QB])
                    # leftover proj pairs (small j windows)
                    if proj_state is not None:
                        while proj_state[2] < 8:
                            emit_proj_pair(proj_state[0], proj_state[1],
                                           proj_state[2])
                            proj_state[2] += 1
                        proj_state = None

                    # evict raw o immediately (ACT cast) so the PSUM banks
                    # free up for the next j's AV accumulation
                    o_raw = p2sb.tile([P, HPC, QB], BF16, tag="oraw", bufs=2,
                                      name=f"oraw_{j}")
                    for h in range(HPC):
                        nc.scalar.copy(o_raw[:, h, :], o_ps[h][:])

                    if j == NJ - 1:
                        o_sb = emit_norm(j, o_raw, acc)
                        for dtp in range(8):
                            emit_proj_pair(j, o_sb, dtp)
                    else:
                        pending = (j, o_raw, acc)
                del rp_late

    nc.compile()
    return nc


_NC = None


def _get_nc():
    global _NC
    if _NC is None:
        _NC = _build()
    return _NC


def _bf16():
    import ml_dtypes
    return ml_dtypes.bfloat16


def _host_inputs(x, vi, Wq, Wk, Wv, Wproj, lamb):
    """Build the per-core input maps (all numpy float32)."""
    x = np.asarray(x, dtype=np.float32).reshape(T, D)
    vi = np.asarray(vi, dtype=np.float32).reshape(T, NH, HD)
    Wq = np.asarray(Wq, dtype=np.float32)
    Wk = np.asarray(Wk, dtype=np.float32)
    Wv = np.asarray(Wv, dtype=np.float32)
    Wproj = np.asarray(Wproj, dtype=np.float32)
    lam = float(np.asarray(lamb))

    # x^T tiled: xt[tt, p, kt, f] = x[tt*P+f, kt*P+p]
    xt = np.ascontiguousarray(
        x.reshape(NT, P, KT, P).transpose(0, 3, 2, 1)).astype(_bf16())

    # rope tables
    inv_freq = (1.0 / 10000.0) ** (np.arange(0, HD, 2, dtype=np.float32) / HD)
    tpos = np.arange(T, dtype=np.float32)
    freqs = np.outer(tpos, inv_freq).astype(np.float32)      # [T, 64]
    cs_full = np.concatenate([np.cos(freqs), np.sin(freqs)], axis=1)  # [T,128]
    cs_t = np.ascontiguousarray(
        cs_full.reshape(NT, P, HD).transpose(1, 0, 2))       # [P, NT, HD]

    bf16 = _bf16()
    # causal triangle mask for the diagonal 128x128 blocks of S^T[k, q]:
    # valid iff q-offset >= k-partition
    tri = (np.arange(P)[:, None] <= np.arange(P)[None, :]).astype(bf16)

    in_maps = []
    for core in range(NCORES):
        r0 = core * HPC * HD
        wq_c = Wq[r0:r0 + HPC * HD]
        wk_c = Wk[r0:r0 + HPC * HD]
        wv_c = Wv[r0:r0 + HPC * HD] * (1.0 - lam)
        w_all = np.concatenate(
            [wq_c[0:HD], wq_c[HD:2 * HD],
             wk_c[0:HD], wk_c[HD:2 * HD],
             wv_c[0:HD], wv_c[HD:2 * HD]], axis=0)           # [768, D]
        # wall[p, kt, m] = w_all[m, kt*P+p]  (W_all^T tiled)
        wall_c = np.ascontiguousarray(
            w_all.reshape(6 * HD, KT, P).transpose(2, 1, 0)).astype(_bf16())
        # wproj[p, ct, m] = Wproj[m, r0 + ct*P + p]
        wp = Wproj[:, r0:r0 + HPC * HD]                       # [D, 256]
        wproj_c = np.ascontiguousarray(
            wp.reshape(D, HPC, P).transpose(2, 1, 0)).astype(bf16)
        # vilam[tt, p, c] = lam * vi[tt*P+p, head, hd]
        vl = (lam * vi[:, HPC * core:HPC * (core + 1), :]).reshape(
            NT, P, HPC * HD)
        in_maps.append({
            "xt": xt,
            "wall": wall_c,
            "wproj": wproj_c,
            "vilam": np.ascontiguousarray(vl),
            "cs": cs_t,
            "tri": tri,
            "identr": np.eye(P, dtype=np.float32).astype(bf16),
        })
    return in_maps


def kernel(x, vi, Wq, Wk, Wv, Wproj, lamb, _trace=False, _trace_kwargs=None):
    nc = _get_nc()
    in_maps = _host_inputs(x, vi, Wq, Wk, Wv, Wproj, lamb)
    res = run_bass_kernel_spmd(nc, in_maps, list(range(NCORES)),
                               trace=_trace, **(_trace_kwargs or {}))
    acc = np.zeros((D, T), dtype=np.float64)
    for core in range(NCORES):
        acc += np.asarray(res.results[core]["out"]).astype(np.float64)
    y = acc.T.astype(np.float32).reshape(1, 1, T, D)
    if _trace:
        return y, res
    return y
